# revision 1
# baseline (speedup 1.0000x reference)
import sys

for _p in ("/opt/trn_rl_repo", "/root/.axon_site/_ro/trn_rl_repo"):
    if _p not in sys.path:
        sys.path.insert(0, _p)

import numpy as np
import ml_dtypes

# Problem constants (nn_LocalConvolution): x [4,256,64,64] f32,
# weight [4,1,16,49,64,64] f32, K=7, pad=3, stride=1, dil=1.
# out[b, g*16+cc, y, x] = sum_k x_pad[b, g*16+cc, y+kh-3, x+kw-3] * w[b,0,cc,k,y,x]
B, C, H, W = 4, 256, 64, 64
WC, KK, K, PAD = 16, 49, 7, 3
NCORES = 8
HHALF = H // 2  # 32 rows per core (B=4 x 2 H-halves = 8 shards)
PART = 128
NBLK = C // PART  # 2 channel blocks per core
XR, XC = HHALF + 2 * PAD, W + 2 * PAD  # 38 x 70 padded shard
BANK = 512  # fp32 elems per PSUM bank
NBANK = HHALF * W // BANK  # 4 banks per block
YB = BANK // W  # 8 y-rows per bank

_BF16 = ml_dtypes.bfloat16
_cache = {}


def _build():
    import concourse.bacc as bacc
    import concourse.mybir as mybir
    import concourse.tile as tile

    nc = bacc.Bacc(None, target_bir_lowering=False)
    bf = mybir.dt.bfloat16
    f32 = mybir.dt.float32

    xa_d = nc.dram_tensor("xa", (PART, NBLK * XR * XC), bf, kind="ExternalInput")
    xb_d = nc.dram_tensor("xb", (PART, NBLK * XR * XC), bf, kind="ExternalInput")
    wr_d = nc.dram_tensor("wr", (PART, KK * HHALF * W), bf, kind="ExternalInput")
    id_d = nc.dram_tensor("ident", (PART, PART), bf, kind="ExternalInput")
    out_d = nc.dram_tensor("out", (PART, NBLK * HHALF * W), f32, kind="ExternalOutput")

    with tile.TileContext(nc) as tc:
        with (
            tc.tile_pool(name="xpool", bufs=1) as xpool,
            tc.tile_pool(name="cpool", bufs=1) as cpool,
            tc.tile_pool(name="wpool", bufs=4) as wpool,
            tc.tile_pool(name="tpool", bufs=8) as tpool,
            tc.tile_pool(name="opool", bufs=2) as opool,
            tc.tile_pool(name="psum", bufs=1, space="PSUM") as ppool,
        ):
            xa_t = xpool.tile([PART, NBLK, XR, XC], bf, tag="xa")
            xb_t = xpool.tile([PART, NBLK, XR, XC], bf, tag="xb")
            id_t = cpool.tile([PART, PART], bf, tag="id")
            nc.sync.dma_start(xa_t[:], xa_d[:])
            nc.sync.dma_start(xb_t[:], xb_d[:])
            nc.sync.dma_start(id_t[:], id_d[:])

            acc = [
                [ppool.tile([PART, BANK], f32, name=f"ps{blk}_{j}", tag=f"ps{blk}_{j}") for j in range(NBANK)]
                for blk in range(NBLK)
            ]

            for kh in range(K):
                for kw in range(K):
                    k = kh * K + kw
                    # per-tap weight DMA (524KB) keeps startup latency low and
                    # interleaves smoothly with compute
                    w_t = wpool.tile([PART, 1, HHALF, W], bf, tag="w")
                    nc.sync.dma_start(
                        w_t[:], wr_d[:, k * HHALF * W : (k + 1) * HHALF * W]
                    )
                    # one DVE mult covers both channel blocks; the weight AP
                    # broadcasts (stride 0) over the block dim
                    tmp = tpool.tile([PART, NBLK, HHALF, W], bf, tag="tmp")
                    if kw % 2 == 0:
                        src = xa_t[:, :, kh : kh + HHALF, kw : kw + W]
                    else:
                        src = xb_t[:, :, kh : kh + HHALF, kw + 1 : kw + 1 + W]
                    wap = w_t[:, 0:1, :, :].broadcast_to((PART, NBLK, HHALF, W))
                    nc.vector.tensor_mul(tmp[:], src, wap)
                    for blk in range(NBLK):
                        for j in range(NBANK):
                            nc.tensor.matmul(
                                acc[blk][j][:],
                                id_t[:],
                                tmp[:, blk, j * YB : (j + 1) * YB, :],
                                start=(k == 0),
                                stop=(k == KK - 1),
                            )

            for blk in range(NBLK):
                for j in range(NBANK):
                    ost = opool.tile([PART, BANK], f32, tag="ost")
                    nc.scalar.copy(ost[:], acc[blk][j][:])
                    nc.sync.dma_start(
                        out_d[:, blk * HHALF * W + j * BANK : blk * HHALF * W + (j + 1) * BANK],
                        ost[:],
                    )

    _dedupe_ldweights(nc)
    nc.compile()
    return nc


def _dedupe_ldweights(nc):
    """All PE matmuls share one identity stationary; drop every InstLdweights
    after the first so the PE array keeps the loaded weights. Only removes
    LdWeights that carry no semaphore activity and whose AP matches the
    first one exactly."""
    first_repr = None
    removed = 0
    for blk in nc.main_func.blocks:
        keep = []
        for inst in blk.instructions:
            if type(inst).__name__ == "InstLdweights":
                si = inst.sync_info
                clean = si is None or (not si.on_wait and not si.on_update)
                r = repr(inst.ins[0])
                if first_repr is None:
                    first_repr = r
                elif clean and r == first_repr:
                    removed += 1
                    continue
            keep.append(inst)
        blk.instructions[:] = keep
    return removed


def _prep_core(x, w, b, h):
    """Host-side shard prep for one core: pad/cast x, slice/replicate w."""
    y0 = h * HHALF
    xa = np.zeros((C, XR, XC), dtype=np.float32)
    ylo, yhi = y0 - PAD, y0 + HHALF + PAD
    slo, shi = max(ylo, 0), min(yhi, H)
    xa[:, slo - ylo : shi - ylo, PAD : PAD + W] = x[b, :, slo:shi, :]
    xb = np.zeros((C, XR, XC), dtype=np.float32)
    xb[:, :, 1:] = xa[:, :, :-1]
    # partition-major: [128, NBLK, XR, XC], channel = blk*128 + p
    xa = xa.reshape(NBLK, PART, XR, XC).transpose(1, 0, 2, 3)
    xb = xb.reshape(NBLK, PART, XR, XC).transpose(1, 0, 2, 3)
    # weights: [128, 49, 32, 64], partition p uses weight channel p % 16
    wsh = w[b, 0, :, :, y0 : y0 + HHALF, :]  # [16, 49, 32, 64]
    wr = np.tile(wsh, (PART // WC, 1, 1, 1))  # [128, 49, 32, 64]
    return (
        xa.reshape(PART, -1).astype(_BF16),
        xb.reshape(PART, -1).astype(_BF16),
        wr.reshape(PART, -1).astype(_BF16),
    )


def kernel(x: np.ndarray, weight: np.ndarray) -> np.ndarray:
    from concourse.bass_utils import run_bass_kernel_spmd

    if "nc" not in _cache:
        _cache["nc"] = _build()
    nc = _cache["nc"]

    ident = np.eye(PART, dtype=_BF16)
    in_maps = []
    for core in range(NCORES):
        b, h = core // 2, core % 2
        xa, xb, wr = _prep_core(x, weight, b, h)
        in_maps.append({"xa": xa, "xb": xb, "wr": wr, "ident": ident})

    res = run_bass_kernel_spmd(nc, in_maps, list(range(NCORES)))

    out = np.empty((B, C, H, W), dtype=np.float32)
    for core in range(NCORES):
        b, h = core // 2, core % 2
        o = res.results[core]["out"].reshape(PART, NBLK, HHALF, W)
        out[b, :, h * HHALF : (h + 1) * HHALF, :] = o.transpose(1, 0, 2, 3).reshape(
            C, HHALF, W
        )
    return out



# revision 2
# speedup vs baseline: 1.0027x; 1.0027x over previous
import sys

for _p in ("/opt/trn_rl_repo", "/root/.axon_site/_ro/trn_rl_repo"):
    if _p not in sys.path:
        sys.path.insert(0, _p)

import numpy as np
import ml_dtypes
from numpy.lib.stride_tricks import sliding_window_view

# Problem constants (nn_LocalConvolution): x [4,256,64,64] f32,
# weight [4,1,16,49,64,64] f32, K=7, pad=3, stride=1, dil=1.
# out[b, g*16+cc, y, x] = sum_k x_pad[b, g*16+cc, y+kh-3, x+kw-3] * w[b,0,cc,k,y,x]
#
# Layout v2: partition p = cc*8 + yo (16 weight-channels x 8 y-octants of a
# 32-row half-image). Each partition holds x windows for all 16 groups g and
# its own 10-row halo window, plus UNREPLICATED weights [kw][kh][yi][x].
# This cuts HBM traffic from 28.4MB to ~8.2MB per core vs the channel-major
# layout (which had to replicate weights 8x across partitions).
B, C, H, W = 4, 256, 64, 64
WC, KK, K, PAD = 16, 49, 7, 3
NCORES = 8
HHALF = H // 2          # 32 rows per core
PART = 128
G = C // WC             # 16 groups sharing each weight channel
NYO = 8                 # y-octants per half (4 rows each)
YI = HHALF // NYO       # 4 rows per octant
NR = YI + 2 * PAD       # 10-row halo window per partition
XC = W + 2 * PAD        # 70 cols (3 zero pad each side)
XELE = G * NR * XC      # 11200 x elems per partition
WELE = K * K * YI * W   # 12544 weight elems per partition
OELE = G * YI * W       # 4096 out elems per partition
BANK = 512              # fp32 elems per PSUM bank; bank j <-> g in {2j, 2j+1}

_BF16 = ml_dtypes.bfloat16
_cache = {}


def _build():
    import concourse.bacc as bacc
    import concourse.mybir as mybir
    import concourse.tile as tile
    from concourse.ap import AP

    nc = bacc.Bacc(None, target_bir_lowering=False)
    bf = mybir.dt.bfloat16

    xa_d = nc.dram_tensor("xa", (PART, XELE), bf, kind="ExternalInput")
    xb_d = nc.dram_tensor("xb", (PART, XELE), bf, kind="ExternalInput")
    wr_d = nc.dram_tensor("wr", (PART, WELE), bf, kind="ExternalInput")
    id_d = nc.dram_tensor("ident", (PART, PART), bf, kind="ExternalInput")
    out_d = nc.dram_tensor("out", (PART, OELE), bf, kind="ExternalOutput")

    def win_ap(t, kh, g0, ng, col_off, nx):
        # [p, g, yi, x] view over an x tile [p, g(700), r(70), c(1)]:
        # row index = kh + yi (overlapping windows), col = col_off + x.
        # DVE ISA allows at most 3 free dims, so ops are per-tap (kh, kw).
        a = t[:]
        return AP(
            a.tensor,
            a.offset + g0 * (NR * XC) + kh * XC + col_off,
            a.ap[0:1] + [[NR * XC, ng], [XC, YI], [1, nx]],
        ).opt()

    def w_ap(t, kh, ng, kw, x0):
        # [p, g(bcast), yi, x] view over weight tile [p, kw, kh, yi, x]
        a = t[:]
        return AP(
            a.tensor,
            a.offset + kw * (K * YI * W) + kh * (YI * W) + x0,
            a.ap[0:1] + [[0, ng], [W, YI], [1, W - x0]],
        ).opt()

    with tile.TileContext(nc) as tc:
        with (
            tc.tile_pool(name="xpool", bufs=1) as xpool,
            tc.tile_pool(name="cpool", bufs=1) as cpool,
            tc.tile_pool(name="tpool", bufs=1) as tpool,
            tc.tile_pool(name="opool", bufs=1) as opool,
            tc.tile_pool(name="psum", bufs=1, space="PSUM") as ppool,
        ):
            xa_t = xpool.tile([PART, G, NR, XC], bf, tag="xa")
            xb_t = xpool.tile([PART, G, NR, XC], bf, tag="xb")
            wt = xpool.tile([PART, K, K, YI, W], bf, tag="wt")
            id_t = cpool.tile([PART, PART], bf, tag="id")
            obuf = opool.tile([PART, G, YI, W], bf, tag="obuf")

            # Single queue, hand-ordered by first-use. Only the (kw0,kh0)
            # weight slice and the first x quarter gate the first multiply;
            # ident is only needed by the first matmul, which trails the
            # multiplies anyway.
            GQ = G // 4  # 4-group x chunks
            xq = G * NR * XC // 4
            kwsz = K * YI * W
            xe = xq // 2
            nc.sync.dma_start(wt[:, 0, 0:1], wr_d[:, 0 : YI * W])
            nc.sync.dma_start(xa_t[:, 0 : GQ // 2], xa_d[:, 0:xe])
            nc.sync.dma_start(xa_t[:, GQ // 2 : GQ], xa_d[:, xe:xq])
            nc.sync.dma_start(wt[:, 0, 1:4], wr_d[:, YI * W : 4 * YI * W])
            nc.sync.dma_start(id_t[:], id_d[:])
            for q in range(1, 4):
                nc.sync.dma_start(
                    xa_t[:, q * GQ : (q + 1) * GQ], xa_d[:, q * xq : (q + 1) * xq]
                )
            nc.sync.dma_start(wt[:, 0, 4:], wr_d[:, 4 * YI * W : kwsz])
            for q in range(2):
                nc.sync.dma_start(
                    xb_t[:, q * GQ : (q + 1) * GQ], xb_d[:, q * xq : (q + 1) * xq]
                )
            nc.sync.dma_start(wt[:, 1], wr_d[:, kwsz : 2 * kwsz])
            for q in range(2, 4):
                nc.sync.dma_start(
                    xb_t[:, q * GQ : (q + 1) * GQ], xb_d[:, q * xq : (q + 1) * xq]
                )
            for kw in range(2, K):
                nc.sync.dma_start(wt[:, kw], wr_d[:, kw * kwsz : (kw + 1) * kwsz])

            acc = [
                ppool.tile([PART, BANK], mybir.dt.float32, name=f"ps{j}", tag=f"ps{j}")
                for j in range(8)
            ]

            # 3 rotating product buffers: with 2, the DVE's write-after-read
            # margin on PE is only ~0.3us and any PE hiccup (cold HAM, late
            # ident) locks the pipeline into a sem-wait-per-op slow mode.
            tmps = [
                tpool.tile([PART, G, YI, W], bf, name=f"tmp{i}", tag=f"tmp{i}")
                for i in range(3)
            ]
            # kw<=1 ops skip output cols 0-1 (their products read zero pad);
            # those tmp cols must BE zero for the full-width matmuls, and stay
            # untouched until kw=2 overwrites them with real products.
            for t in tmps:
                nc.scalar.memzero(t[:, :, :, 0:2])
            opidx = [0]

            def mults(kw, kh, g0, ng):
                src_t = xa_t if kw % 2 == 0 else xb_t
                col_off = kw if kw % 2 == 0 else kw + 1
                x0 = 2 if kw <= 1 else 0
                tmp = tmps[opidx[0] % 3]
                opidx[0] += 1
                nc.vector.tensor_mul(
                    tmp[:, g0 : g0 + ng, :, x0:W].opt(),
                    win_ap(src_t, kh, g0, ng, col_off + x0, W - x0),
                    w_ap(wt, kh, ng, kw, x0),
                )
                first = kw == 0 and kh == 0
                last = kw == K - 1 and kh == K - 1
                for gg in range(g0, g0 + ng, 2):
                    j = gg // 2
                    nc.tensor.matmul(
                        acc[j][:], id_t[:], tmp[:, gg : gg + 2], start=first, stop=last
                    )

            # kw=0 kh=0-2 run per g-quarter (q-outer) so compute tracks x
            # chunk arrival; the rest runs full-width per tap (short ops pay
            # a ~230ns drain that full ops amortize). The very last tap runs
            # as g-halves so PSUM banks 0-3 drain to HBM while 4-7 compute.
            mults(0, 0, 0, GQ // 2)
            mults(0, 0, GQ // 2, GQ // 2)
            for kh in range(1, 4):
                mults(0, kh, 0, GQ)
            for q in range(1, 4):
                for kh in range(4):
                    mults(0, kh, q * GQ, GQ)
            for kh in range(4, K):
                mults(0, kh, 0, G)
            for kw in range(1, K):
                for kh in range(K):
                    if kw == K - 1 and kh == K - 1:
                        mults(kw, kh, 0, G // 2)
                        mults(kw, kh, G // 2, G // 2)
                    else:
                        mults(kw, kh, 0, G)

            # Output drain, emitted after the final multiply so the DVE-side
            # copies never block a pending tensor_mul in queue order. Copies
            # alternate ACT/DVE (parallel PSUM reads on different banks).
            for j in range(8):
                gg = 2 * j
                if j % 2 == 0:
                    nc.scalar.copy(obuf[:, gg : gg + 2], acc[j][:])
                else:
                    nc.vector.tensor_copy(obuf[:, gg : gg + 2], acc[j][:])
                if j == 3:
                    nc.sync.dma_start(out_d[:, : OELE // 2], obuf[:, : G // 2])
                elif j == 7:
                    nc.sync.dma_start(out_d[:, OELE // 2 :], obuf[:, G // 2 :])

    _dedupe_ldweights(nc)
    nc.compile()
    return nc


def _dedupe_ldweights(nc):
    """All PE matmuls share one identity stationary; drop every InstLdweights
    after the first so the PE array keeps the loaded weights."""
    first_repr = None
    removed = 0
    for blk in nc.main_func.blocks:
        keep = []
        for inst in blk.instructions:
            if type(inst).__name__ == "InstLdweights":
                si = inst.sync_info
                clean = si is None or (not si.on_wait and not si.on_update)
                r = repr(inst.ins[0])
                if first_repr is None:
                    first_repr = r
                elif clean and r == first_repr:
                    removed += 1
                    continue
            keep.append(inst)
        blk.instructions[:] = keep
    return removed


def _prep_core(x, w, b, h):
    """Host-side shard prep for one core: build per-partition halo windows of
    x (and a 1-col-shifted copy for odd kw alignment) plus unreplicated
    weights in [cc*8+yo][kw][kh][yi][x] order."""
    y0 = h * HHALF
    xa = np.zeros((C, 38, XC), dtype=np.float32)
    ylo, yhi = y0 - PAD, y0 + HHALF + PAD + 4 - 3  # rows y0-3 .. y0+35
    slo, shi = max(ylo, 0), min(y0 + 35, H)
    xa[:, slo - ylo : shi - ylo, PAD : PAD + W] = x[b, :, slo:shi, :]
    xbf = np.zeros_like(xa)
    xbf[:, :, 1:] = xa[:, :, :-1]

    def windows(arr):
        wv = sliding_window_view(arr, NR, axis=1)  # [C, 29, 70, 10]
        wv = wv[:, 0 : 4 * NYO : 4]  # 8 octant windows
        wv = wv.transpose(0, 1, 3, 2)  # [C, yo, r, c]
        wv = wv.reshape(G, WC, NYO, NR, XC).transpose(1, 2, 0, 3, 4)
        return np.ascontiguousarray(wv).reshape(PART, XELE)

    ws = w[b, 0, :, :, y0 : y0 + HHALF, :]  # [16, 49, 32, 64]
    ws = ws.reshape(WC, K, K, NYO, YI, W)  # [cc, kh, kw, yo, yi, x]
    ws = ws.transpose(0, 3, 2, 1, 4, 5)  # [cc, yo, kw, kh, yi, x]
    wr = np.ascontiguousarray(ws).reshape(PART, WELE)
    return (
        windows(xa).astype(_BF16),
        windows(xbf).astype(_BF16),
        wr.astype(_BF16),
    )


def kernel(x: np.ndarray, weight: np.ndarray) -> np.ndarray:
    from concourse.bass_utils import run_bass_kernel_spmd

    if "nc" not in _cache:
        _cache["nc"] = _build()
    nc = _cache["nc"]

    ident = np.eye(PART, dtype=_BF16)
    in_maps = []
    for core in range(NCORES):
        b, h = core // 2, core % 2
        xa, xb, wr = _prep_core(x, weight, b, h)
        in_maps.append({"xa": xa, "xb": xb, "wr": wr, "ident": ident})

    res = run_bass_kernel_spmd(nc, in_maps, list(range(NCORES)))

    out = np.empty((B, C, H, W), dtype=np.float32)
    for core in range(NCORES):
        b, h = core // 2, core % 2
        o = np.asarray(res.results[core]["out"]).astype(np.float32)
        o = o.reshape(WC, NYO, G, YI, W).transpose(2, 0, 1, 3, 4)
        out[b, :, h * HHALF : (h + 1) * HHALF, :] = o.reshape(C, HHALF, W)
    return out
